# revision 11
# baseline (speedup 1.0000x reference)
"""GCNConv (N=100000 nodes, d=64, E=1.6M edges) on 8 Trainium2 NeuronCores.

Formula (DGL GraphConv, in==out feats):
    out_deg = bincount(src); in_deg = bincount(dst)
    norm_src = clip(out_deg,1)^-0.5 ; norm_dst = clip(in_deg,1)^-0.5
    feat = x * norm_src[:,None]
    agg[d] = sum_{e: dst[e]=d} feat[src[e]]
    out = (agg * norm_dst[:,None]) @ W

Distribution: nodes sharded 8 ways (12500/core, padded to 12544 = 128*98).
Host prep is pure edge-index work: global CSR rowptrs (src-/dst-sorted edge
offsets), per-core (dst-window x src-quarter) edge buckets, int16 gather
index buffers.

  Phase 1 (per core): degrees from rowptr diffs on device (sub, clip,
    rsqrt); one [128, 98, 64] multiply scales the x shard into a bf16
    feature table shard (rows padded to 128 cols for 256 B gather elems).
  AllGather feat shards -> full gather table [100352, 128] bf16 per core.
  Phase 2 (core k; edges with dst in shard k, bucketed by (src-quarter q,
    dst-window w), quarter-major): big dma_gather calls (up to 64 tiles =
    8192 rows per gpsimd instruction; int16 idx limit forces 4 base-offset
    quarters); per 128-edge tile a one-hot scatter matmul accumulates into
    PSUM aggT [64, 128] per (q, w) segment; segments of the same window
    combine in f32 SBUF accumulators (scalar engine); per window:
    out_blk = aggT.T @ W, row-scale by norm_dst, DMA out.
"""

import sys

if "/opt/trn_rl_repo" not in sys.path:
    sys.path.insert(0, "/opt/trn_rl_repo")

import numpy as np

import concourse.bass as bass
import concourse.mybir as mybir
import concourse.tile as tile

N_NODES = 100000
D = 64
N_CORES = 8
SHARD = N_NODES // N_CORES  # 12500
P = 128  # edges per tile (matmul contraction dim)
W2 = 128  # dst window width == node block
NW = 98  # windows (= 128-node blocks) per core; 128*98 = 12544
SHARD_PAD = P * NW  # 12544
NFULL = SHARD_PAD * N_CORES  # 100352
EPAD = 128  # padded feature row length (256 B)
QS = 32768  # gather quarter size (int16 index limit)
NQ = 4  # quarters
CHG = 8  # max tiles per dma_gather call (1024 idxs; SWDGE ring cap)
CHO = 32  # tiles per one-hot chunk

F32 = mybir.dt.float32
BF16 = mybir.dt.bfloat16
I32 = mybir.dt.int32
I16 = mybir.dt.int16


def split_waits(nc, maxw=1):
    """This walrus build allows at most `maxw` sem-waits per instruction;
    move extras onto preceding InstEventSemaphore carriers (same engine)."""
    for f in nc.m.functions:
        for blk in f.blocks:
            newl = []
            for ins in blk.instructions:
                si = ins.sync_info
                if si is not None and si.on_wait and len(si.on_wait) > maxw:
                    waits = list(si.on_wait)
                    carry, keep = waits[:-maxw], waits[-maxw:]
                    for i in range(0, len(carry), maxw):
                        w = mybir.InstEventSemaphore(
                            name=nc.get_next_instruction_name(), ins=[], outs=[]
                        )
                        w.engine = ins.engine
                        w.sync_info = mybir.SyncInfo(
                            on_wait=carry[i : i + maxw], on_update=[]
                        )
                        newl.append(w)
                    ins.sync_info = mybir.SyncInfo(
                        on_wait=keep, on_update=list(si.on_update)
                    )
                newl.append(ins)
            blk.instructions[:] = newl


def _prep(x, W, src, dst):
    """Host-side sharding: CSR rowptrs, per-core (quarter, window) edge
    buckets, gather index buffers, and the shared tile map."""
    import ml_dtypes

    src = np.asarray(src)
    dst = np.asarray(dst)
    x = np.asarray(x, dtype=np.float32)
    W = np.asarray(W, dtype=np.float32)

    rp_src = np.zeros(N_NODES + 1, dtype=np.int64)
    np.cumsum(np.bincount(src, minlength=N_NODES), out=rp_src[1:])
    rp_dst = np.zeros(N_NODES + 1, dtype=np.int64)
    np.cumsum(np.bincount(dst, minlength=N_NODES), out=rp_dst[1:])

    order = np.argsort(dst, kind="stable")
    dst_sorted = dst[order]
    src_by_dst = src[order]

    cqw = np.zeros((N_CORES, NQ * NW), dtype=np.int64)
    per_core = []
    for k in range(N_CORES):
        lo, hi = rp_dst[SHARD * k], rp_dst[SHARD * (k + 1)]
        loc = dst_sorted[lo:hi] - SHARD * k
        gsrc = src_by_dst[lo:hi]
        gadj = (gsrc // SHARD) * SHARD_PAD + (gsrc % SHARD)
        wv = loc // W2
        qv = gadj // QS
        key = qv * NW + wv
        cqw[k] = np.bincount(key, minlength=NQ * NW)
        per_core.append((loc, wv, qv, key, gadj))

    t_qw = ((cqw.max(axis=0) + P - 1) // P).astype(np.int64)  # [NQ*NW]
    t_base = np.concatenate([[0], np.cumsum(t_qw)[:-1]])
    T2 = int(t_qw.sum())

    bf16 = ml_dtypes.bfloat16
    w64 = W.astype(bf16)
    iota = np.broadcast_to(np.arange(W2, dtype=np.float32), (P, W2)).astype(bf16)

    # phase-1 node layout: local id l = NW*p + b  (partition-contiguous DMA)
    lgridS = np.arange(P)[:, None] * NW + np.arange(NW)[None, :]
    validS = lgridS < SHARD
    # phase-2 / output node layout: local id l = W2*w + p
    lgridD = np.arange(P)[:, None] + W2 * np.arange(NW)[None, :]
    validD = lgridD < SHARD

    ins_maps = []
    for k in range(N_CORES):
        loc, wv, qv, key, gadj = per_core[k]
        # order edges by (quarter, window, gadj) for gather locality
        eorder = np.lexsort((gadj, key))
        keyo = key[eorder]
        loco = loc[eorder]
        gadjo = gadj[eorder]
        qvo = qv[eorder]

        starts = np.concatenate([[0], np.cumsum(np.bincount(keyo, minlength=NQ * NW))[:-1]])
        rank = np.arange(len(keyo)) - starts[keyo]
        tcol = (t_base[keyo] + rank // P).astype(np.int64)
        lane = (rank % P).astype(np.int64)

        p2win = np.full((P, T2), float(W2), dtype=np.float32)
        p2win[lane, tcol] = (loco - W2 * (keyo % NW)).astype(np.float32)
        qidx = np.zeros((16, 8 * T2), dtype=np.int16)
        qidx[lane % 16, 8 * tcol + lane // 16] = (gadjo - QS * qvo).astype(np.int16)
        qidx = np.tile(qidx, (8, 1))  # replicate across the 8 Q7 cores

        n0 = SHARD * k
        gS = n0 + np.minimum(lgridS, SHARD - 1)
        posS0 = np.where(validS, rp_src[gS], 0).astype(np.float32)
        posS1 = np.where(validS, rp_src[gS + 1], 1).astype(np.float32)
        gD = n0 + np.minimum(lgridD, SHARD - 1)
        posD0 = np.where(validD, rp_dst[gD], 0).astype(np.float32)
        posD1 = np.where(validD, rp_dst[gD + 1], 1).astype(np.float32)

        xs = np.zeros((SHARD_PAD, D), dtype=np.float32)
        xs[:SHARD] = x[n0 : n0 + SHARD]

        ins_maps.append(
            {
                "xs": np.ascontiguousarray(xs.reshape(P, NW, D)),
                "posS0": posS0,
                "posS1": posS1,
                "posD0": posD0,
                "posD1": posD1,
                "qidx": np.ascontiguousarray(qidx),
                "p2win": p2win.astype(bf16),
                "w64": w64,
                "iota": iota,
            }
        )

    meta = {"T2": T2, "t_qw": t_qw}
    return ins_maps, meta


def _tile_maps(meta):
    t_qw = meta["t_qw"]
    win_of_tile = []
    q_of_tile = []
    seg_first = {}
    seg_last = {}
    segs_of_win = {w: [] for w in range(NW)}
    for q in range(NQ):
        for w in range(NW):
            n = int(t_qw[q * NW + w])
            if n == 0:
                continue
            t0 = len(win_of_tile)
            seg_first[(q, w)] = t0
            seg_last[(q, w)] = t0 + n - 1
            segs_of_win[w].append(q)
            win_of_tile.extend([w] * n)
            q_of_tile.extend([q] * n)
    T2 = len(win_of_tile)
    assert T2 == meta["T2"]

    # gather chunks: runs of <= CHG tiles within one quarter
    chunks = []
    t = 0
    while t < T2:
        q = q_of_tile[t]
        ch = 1
        while ch < CHG and t + ch < T2 and q_of_tile[t + ch] == q:
            ch += 1
        chunks.append((t, ch, q))
        t += ch

    meta["win_of_tile"] = win_of_tile
    meta["q_of_tile"] = q_of_tile
    meta["seg_first"] = seg_first
    meta["seg_last"] = seg_last
    meta["segs_of_win"] = segs_of_win
    meta["chunks"] = chunks
    meta["empty_wins"] = [w for w in range(NW) if not segs_of_win[w]]
    return meta


def _build_nc(meta, do_split_waits=True):
    from concourse import library_config

    T2 = meta["T2"]

    nc = bass.Bass(num_swdge_queues=4)
    xs = nc.declare_dram_parameter("xs", [P, NW, D], F32, isOutput=False)
    posS0_d = nc.declare_dram_parameter("posS0", [P, NW], F32, isOutput=False)
    posS1_d = nc.declare_dram_parameter("posS1", [P, NW], F32, isOutput=False)
    posD0_d = nc.declare_dram_parameter("posD0", [P, NW], F32, isOutput=False)
    posD1_d = nc.declare_dram_parameter("posD1", [P, NW], F32, isOutput=False)
    qidx_d = nc.declare_dram_parameter("qidx", [P, 8 * T2], I16, isOutput=False)
    p2win_d = nc.declare_dram_parameter("p2win", [P, T2], BF16, isOutput=False)
    w64_d = nc.declare_dram_parameter("w64", [D, D], BF16, isOutput=False)
    iota_d = nc.declare_dram_parameter("iota", [P, W2], BF16, isOutput=False)
    out_d = nc.declare_dram_parameter("out", [SHARD, D], F32, isOutput=True)

    feat_s = nc.dram_tensor("feat_s", [P, NW, EPAD], BF16)
    feat_f = nc.dram_tensor("feat_f", [NFULL, EPAD], BF16)

    with tile.TileContext(nc) as tc:
        with tc.tile_critical():
            nc.gpsimd.load_library(library_config.mlp)
        with tc.tile_pool(name="consts", bufs=1) as consts:
            w64_sb = consts.tile([D, D], BF16, tag="w64")
            iota_sb = consts.tile([P, W2], BF16, tag="iota")
            normD = consts.tile([P, NW], F32, tag="normD")
            acc = consts.tile([D, NW, P], F32, tag="acc")
            qidx_sb = consts.tile([P, 8 * T2], I16, tag="qidx")
            p2win_sb = consts.tile([P, T2], BF16, tag="p2win")
            nc.sync.dma_start(out=w64_sb[:], in_=w64_d[:])
            nc.sync.dma_start(out=iota_sb[:], in_=iota_d[:])
            nc.sync.dma_start(out=qidx_sb[:], in_=qidx_d[:])
            nc.sync.dma_start(out=p2win_sb[:], in_=p2win_d[:])

            # ---------------- phase 1: norms + feat table shard -------------
            with tc.tile_pool(name="p1", bufs=1) as p1:
                pS0 = p1.tile([P, NW], F32, tag="pS0")
                pS1 = p1.tile([P, NW], F32, tag="pS1")
                pD0 = p1.tile([P, NW], F32, tag="pD0")
                pD1 = p1.tile([P, NW], F32, tag="pD1")
                nc.sync.dma_start(out=pS0[:], in_=posS0_d[:])
                nc.sync.dma_start(out=pS1[:], in_=posS1_d[:])
                nc.sync.dma_start(out=pD0[:], in_=posD0_d[:])
                nc.sync.dma_start(out=pD1[:], in_=posD1_d[:])

                def rsqrt_diff(dstt, a1, a0, pool, pfx):
                    dg = pool.tile([P, NW], F32, tag=pfx + "dg")
                    nc.vector.tensor_tensor(
                        out=dg[:], in0=a1[:], in1=a0[:],
                        op=mybir.AluOpType.subtract,
                    )
                    cl = pool.tile([P, NW], F32, tag=pfx + "cl")
                    nc.vector.tensor_scalar_max(cl[:], dg[:], 1.0)
                    sq = pool.tile([P, NW], F32, tag=pfx + "sq")
                    nc.scalar.sqrt(sq[:], cl[:])
                    nc.vector.reciprocal(dstt[:], sq[:])

                normS = p1.tile([P, NW], F32, tag="normS")
                rsqrt_diff(normS, pS1, pS0, p1, "s")
                rsqrt_diff(normD, pD1, pD0, p1, "d")

                xb = p1.tile([P, NW, D], F32, tag="xb")
                nc.sync.dma_start(out=xb[:], in_=xs[:])
                fb = p1.tile([P, NW, EPAD], BF16, tag="fb")
                nc.vector.memset(fb[:, :, D:EPAD], 0.0)
                nc.vector.tensor_mul(
                    fb[:, :, 0:D], xb[:], normS[:, :, None].to_broadcast([P, NW, D])
                )
                nc.sync.dma_start(out=feat_s[:], in_=fb[:])

            # ---------------- allgather feat --------------------------------
            # Completion fence: Tile doesn't track the collective->gather RAW
            # dep through DRAM, so wait on an explicit semaphore inside a
            # critical section (Pool program order covers later gathers).
            ccsem = nc.alloc_semaphore("ccsem")
            with tc.tile_critical():
                nc.gpsimd.collective_compute(
                    "AllGather",
                    mybir.AluOpType.bypass,
                    replica_groups=[list(range(N_CORES))],
                    ins=[feat_s[:]],
                    outs=[feat_f[:]],
                ).then_inc(ccsem, 1)
                nc.gpsimd.wait_ge(ccsem, 1)

            # -------- phase 2: chunked gather + scatter matmul + W ----------
            with (
                tc.tile_pool(name="p2g", bufs=8) as p_g,
                tc.tile_pool(name="p2oh", bufs=3) as p_oh,
                tc.tile_pool(name="p2ps", bufs=4, space="PSUM") as p_ps,
                tc.tile_pool(name="p2ops", bufs=2, space="PSUM") as p_ops,
                tc.tile_pool(name="p2ag", bufs=2) as p_ag,
                tc.tile_pool(name="p2ob", bufs=2) as p_ob,
            ):

                def epilogue(w, ag):
                    op = p_ops.tile([P, D], F32)
                    nc.tensor.matmul(
                        out=op[:], lhsT=ag[:], rhs=w64_sb[:], start=True, stop=True
                    )
                    ob = p_ob.tile([P, D], F32, tag="ob")
                    nc.vector.tensor_mul(
                        ob[:], op[:], normD[:, w : w + 1].to_broadcast([P, D])
                    )
                    nb = min(P, SHARD - P * w)
                    nc.sync.dma_start(
                        out=out_d[P * w : P * w + nb, :], in_=ob[:nb, :]
                    )

                chunks = meta["chunks"]
                nidx_regs = {}
                ci = -1  # current chunk idx
                gb = None
                g0 = 0
                oh = None
                o0 = 0
                ps = None
                for t in range(T2):
                    if ci + 1 < len(chunks) and chunks[ci + 1][0] == t:
                        ci += 1
                        g0, ch, q = chunks[ci]
                        gb = p_g.tile([P, ch, EPAD], BF16, tag="gb")
                        qrows = min(QS, NFULL - QS * q)
                        n = P * ch
                        if n not in nidx_regs:
                            nidx_regs[n] = nc.gpsimd.to_reg(n)
                        nc.gpsimd.dma_gather(
                            out_ap=gb[:],
                            in_ap=feat_f[QS * q : QS * q + qrows, :],
                            idxs_ap=qidx_sb[:, 8 * g0 : 8 * (g0 + ch)],
                            num_idxs=n,
                            num_idxs_reg=nidx_regs[n],
                            elem_size=EPAD,
                            queue_num=ci % 4,
                        )
                    if t % CHO == 0:
                        o0 = t
                        co = min(CHO, T2 - t)
                        oh = p_oh.tile([P, co, W2], BF16, tag="oh")
                        nc.vector.tensor_tensor(
                            out=oh[:],
                            in0=p2win_sb[:, t : t + co, None].to_broadcast([P, co, W2]),
                            in1=iota_sb[:, None, :].to_broadcast([P, co, W2]),
                            op=mybir.AluOpType.is_equal,
                        )
                    w = meta["win_of_tile"][t]
                    q = meta["q_of_tile"][t]
                    if ps is None:
                        ps = p_ps.tile([D, P], F32)
                    nc.tensor.matmul(
                        out=ps[:],
                        lhsT=gb[:, t - g0, 0:D],
                        rhs=oh[:, t - o0, :],
                        start=(t == meta["seg_first"][(q, w)]),
                        stop=(t == meta["seg_last"][(q, w)]),
                    )
                    if t == meta["seg_last"][(q, w)]:
                        segs = meta["segs_of_win"][w]
                        if len(segs) == 1:
                            ag = p_ag.tile([D, P], BF16, tag="ag")
                            nc.vector.tensor_copy(ag[:], ps[:])
                            epilogue(w, ag)
                        elif q == segs[0]:
                            nc.vector.tensor_copy(acc[:, w, :], ps[:])
                        elif q != segs[-1]:
                            nc.vector.tensor_add(acc[:, w, :], ps[:], acc[:, w, :])
                        else:
                            ag = p_ag.tile([D, P], BF16, tag="ag")
                            nc.vector.tensor_add(ag[:], ps[:], acc[:, w, :])
                            epilogue(w, ag)
                        ps = None
                for w in meta["empty_wins"]:
                    ag = p_ag.tile([D, P], BF16, tag="ag")
                    nc.vector.memset(ag[:], 0.0)
                    epilogue(w, ag)

    from concourse.library_overlay import lower_extended_insts

    lower_extended_insts(nc)
    if do_split_waits:
        split_waits(nc)
    return nc


def kernel(x, W, src, dst):
    from concourse.bass_utils import run_bass_kernel_spmd

    ins_maps, meta = _prep(x, W, src, dst)
    meta = _tile_maps(meta)
    nc = _build_nc(meta)
    res = run_bass_kernel_spmd(nc, ins_maps, list(range(N_CORES)))
    out = np.concatenate([res.results[k]["out"] for k in range(N_CORES)], axis=0)
    return out.astype(np.float32)


# revision 13
# speedup vs baseline: 1.0331x; 1.0331x over previous
"""GCNConv (N=100000 nodes, d=64, E=1.6M edges) on 8 Trainium2 NeuronCores.

Formula (DGL GraphConv, in==out feats):
    out_deg = bincount(src); in_deg = bincount(dst)
    norm_src = clip(out_deg,1)^-0.5 ; norm_dst = clip(in_deg,1)^-0.5
    feat = x * norm_src[:,None]
    agg[d] = sum_{e: dst[e]=d} feat[src[e]]
    out = (agg * norm_dst[:,None]) @ W

Distribution: nodes sharded 8 ways (12500/core, padded to 12544 = 128*98).
Host prep is pure edge-index work: global CSR rowptrs (src-/dst-sorted edge
offsets), per-core (dst-window x src-quarter) edge buckets, int16 gather
index buffers.

  Phase 1 (per core): degrees from rowptr diffs on device (sub, clip,
    rsqrt); one [128, 98, 64] multiply scales the x shard into a bf16
    feature table shard (rows padded to 128 cols for 256 B gather elems).
  AllGather feat shards -> full gather table [100352, 128] bf16 per core.
  Phase 2 (core k; edges with dst in shard k, bucketed by (src-quarter q,
    dst-window w), quarter-major): big dma_gather calls (up to 64 tiles =
    8192 rows per gpsimd instruction; int16 idx limit forces 4 base-offset
    quarters); per 128-edge tile a one-hot scatter matmul accumulates into
    PSUM aggT [64, 128] per (q, w) segment; segments of the same window
    combine in f32 SBUF accumulators (scalar engine); per window:
    out_blk = aggT.T @ W, row-scale by norm_dst, DMA out.
"""

import sys

if "/opt/trn_rl_repo" not in sys.path:
    sys.path.insert(0, "/opt/trn_rl_repo")

import numpy as np

import concourse.bass as bass
import concourse.mybir as mybir
import concourse.tile as tile

N_NODES = 100000
D = 64
N_CORES = 8
SHARD = N_NODES // N_CORES  # 12500
P = 128  # edges per tile (matmul contraction dim)
W2 = 128  # dst window width == node block
NW = 98  # windows (= 128-node blocks) per core; 128*98 = 12544
SHARD_PAD = P * NW  # 12544
NFULL = SHARD_PAD * N_CORES  # 100352
EPAD = 128  # padded feature row length (256 B)
QS = 32768  # gather quarter size (int16 index limit)
NQ = 4  # quarters
CHG = 8  # max tiles per dma_gather call (1024 idxs; SWDGE ring cap)
CHO = 32  # tiles per one-hot chunk

F32 = mybir.dt.float32
BF16 = mybir.dt.bfloat16
I32 = mybir.dt.int32
I16 = mybir.dt.int16


def split_waits(nc, maxw=1):
    """This walrus build allows at most `maxw` sem-waits per instruction;
    move extras onto preceding InstEventSemaphore carriers (same engine)."""
    for f in nc.m.functions:
        for blk in f.blocks:
            newl = []
            for ins in blk.instructions:
                si = ins.sync_info
                if si is not None and si.on_wait and len(si.on_wait) > maxw:
                    waits = list(si.on_wait)
                    carry, keep = waits[:-maxw], waits[-maxw:]
                    for i in range(0, len(carry), maxw):
                        w = mybir.InstEventSemaphore(
                            name=nc.get_next_instruction_name(), ins=[], outs=[]
                        )
                        w.engine = ins.engine
                        w.sync_info = mybir.SyncInfo(
                            on_wait=carry[i : i + maxw], on_update=[]
                        )
                        newl.append(w)
                    ins.sync_info = mybir.SyncInfo(
                        on_wait=keep, on_update=list(si.on_update)
                    )
                newl.append(ins)
            blk.instructions[:] = newl


def _prep(x, W, src, dst):
    """Host-side sharding: CSR rowptrs, per-core (quarter, window) edge
    buckets, gather index buffers, and the shared tile map."""
    import ml_dtypes

    src = np.asarray(src)
    dst = np.asarray(dst)
    x = np.asarray(x, dtype=np.float32)
    W = np.asarray(W, dtype=np.float32)

    rp_src = np.zeros(N_NODES + 1, dtype=np.int64)
    np.cumsum(np.bincount(src, minlength=N_NODES), out=rp_src[1:])
    rp_dst = np.zeros(N_NODES + 1, dtype=np.int64)
    np.cumsum(np.bincount(dst, minlength=N_NODES), out=rp_dst[1:])

    order = np.argsort(dst, kind="stable")
    dst_sorted = dst[order]
    src_by_dst = src[order]

    cqw = np.zeros((N_CORES, NQ * NW), dtype=np.int64)
    per_core = []
    for k in range(N_CORES):
        lo, hi = rp_dst[SHARD * k], rp_dst[SHARD * (k + 1)]
        loc = dst_sorted[lo:hi] - SHARD * k
        gsrc = src_by_dst[lo:hi]
        gadj = (gsrc // SHARD) * SHARD_PAD + (gsrc % SHARD)
        wv = loc // W2
        qv = gadj // QS
        key = qv * NW + wv
        cqw[k] = np.bincount(key, minlength=NQ * NW)
        per_core.append((loc, wv, qv, key, gadj))

    t_qw = ((cqw.max(axis=0) + P - 1) // P).astype(np.int64)  # [NQ*NW]
    t_base = np.concatenate([[0], np.cumsum(t_qw)[:-1]])
    T2 = int(t_qw.sum())

    bf16 = ml_dtypes.bfloat16
    w64 = W.astype(bf16)
    iota = np.broadcast_to(np.arange(W2, dtype=np.float32), (P, W2)).astype(bf16)

    # phase-1 node layout: local id l = NW*p + b  (partition-contiguous DMA)
    lgridS = np.arange(P)[:, None] * NW + np.arange(NW)[None, :]
    validS = lgridS < SHARD
    # phase-2 / output node layout: local id l = W2*w + p
    lgridD = np.arange(P)[:, None] + W2 * np.arange(NW)[None, :]
    validD = lgridD < SHARD

    ins_maps = []
    for k in range(N_CORES):
        loc, wv, qv, key, gadj = per_core[k]
        # order edges by (quarter, window, gadj) for gather locality
        eorder = np.lexsort((gadj, key))
        keyo = key[eorder]
        loco = loc[eorder]
        gadjo = gadj[eorder]
        qvo = qv[eorder]

        starts = np.concatenate([[0], np.cumsum(np.bincount(keyo, minlength=NQ * NW))[:-1]])
        rank = np.arange(len(keyo)) - starts[keyo]
        tcol = (t_base[keyo] + rank // P).astype(np.int64)
        lane = (rank % P).astype(np.int64)

        p2win = np.full((P, T2), float(W2), dtype=np.float32)
        p2win[lane, tcol] = (loco - W2 * (keyo % NW)).astype(np.float32)
        qidx = np.zeros((16, 8 * T2), dtype=np.int16)
        qidx[lane % 16, 8 * tcol + lane // 16] = (gadjo - QS * qvo).astype(np.int16)
        qidx = np.tile(qidx, (8, 1))  # replicate across the 8 Q7 cores

        n0 = SHARD * k
        gS = n0 + np.minimum(lgridS, SHARD - 1)
        posS0 = np.where(validS, rp_src[gS], 0).astype(np.float32)
        posS1 = np.where(validS, rp_src[gS + 1], 1).astype(np.float32)
        gD = n0 + np.minimum(lgridD, SHARD - 1)
        posD0 = np.where(validD, rp_dst[gD], 0).astype(np.float32)
        posD1 = np.where(validD, rp_dst[gD + 1], 1).astype(np.float32)

        xs = np.zeros((SHARD_PAD, D), dtype=np.float32)
        xs[:SHARD] = x[n0 : n0 + SHARD]

        ins_maps.append(
            {
                "xs": np.ascontiguousarray(xs.reshape(P, NW, D)),
                "posS0": posS0,
                "posS1": posS1,
                "posD0": posD0,
                "posD1": posD1,
                "qidx": np.ascontiguousarray(qidx),
                "p2win": p2win.astype(bf16),
                "w64": w64,
                "iota": iota,
            }
        )

    meta = {"T2": T2, "t_qw": t_qw}
    return ins_maps, meta


def _tile_maps(meta):
    t_qw = meta["t_qw"]
    win_of_tile = []
    q_of_tile = []
    seg_first = {}
    seg_last = {}
    segs_of_win = {w: [] for w in range(NW)}
    for q in range(NQ):
        for w in range(NW):
            n = int(t_qw[q * NW + w])
            if n == 0:
                continue
            t0 = len(win_of_tile)
            seg_first[(q, w)] = t0
            seg_last[(q, w)] = t0 + n - 1
            segs_of_win[w].append(q)
            win_of_tile.extend([w] * n)
            q_of_tile.extend([q] * n)
    T2 = len(win_of_tile)
    assert T2 == meta["T2"]

    # gather chunks: runs of <= CHG tiles within one quarter
    chunks = []
    t = 0
    while t < T2:
        q = q_of_tile[t]
        ch = 1
        while ch < CHG and t + ch < T2 and q_of_tile[t + ch] == q:
            ch += 1
        chunks.append((t, ch, q))
        t += ch

    meta["win_of_tile"] = win_of_tile
    meta["q_of_tile"] = q_of_tile
    meta["seg_first"] = seg_first
    meta["seg_last"] = seg_last
    meta["segs_of_win"] = segs_of_win
    meta["chunks"] = chunks
    meta["empty_wins"] = [w for w in range(NW) if not segs_of_win[w]]
    return meta


def _build_nc(meta, do_split_waits=True):
    from concourse import library_config

    T2 = meta["T2"]

    nc = bass.Bass(num_swdge_queues=4)
    xs = nc.declare_dram_parameter("xs", [P, NW, D], F32, isOutput=False)
    posS0_d = nc.declare_dram_parameter("posS0", [P, NW], F32, isOutput=False)
    posS1_d = nc.declare_dram_parameter("posS1", [P, NW], F32, isOutput=False)
    posD0_d = nc.declare_dram_parameter("posD0", [P, NW], F32, isOutput=False)
    posD1_d = nc.declare_dram_parameter("posD1", [P, NW], F32, isOutput=False)
    qidx_d = nc.declare_dram_parameter("qidx", [P, 8 * T2], I16, isOutput=False)
    p2win_d = nc.declare_dram_parameter("p2win", [P, T2], BF16, isOutput=False)
    w64_d = nc.declare_dram_parameter("w64", [D, D], BF16, isOutput=False)
    iota_d = nc.declare_dram_parameter("iota", [P, W2], BF16, isOutput=False)
    out_d = nc.declare_dram_parameter("out", [SHARD, D], F32, isOutput=True)

    feat_s = nc.dram_tensor("feat_s", [P, NW, EPAD], BF16)
    feat_f = nc.dram_tensor("feat_f", [NFULL, EPAD], BF16)

    with tile.TileContext(nc) as tc:
        with tc.tile_critical():
            nc.gpsimd.load_library(library_config.mlp)
        with tc.tile_pool(name="consts", bufs=1) as consts:
            w64_sb = consts.tile([D, D], BF16, tag="w64")
            iota_sb = consts.tile([P, W2], BF16, tag="iota")
            normD = consts.tile([P, NW], F32, tag="normD")
            acc = consts.tile([D, NW, P], F32, tag="acc")
            qidx_sb = consts.tile([P, 8 * T2], I16, tag="qidx")
            p2win_sb = consts.tile([P, T2], BF16, tag="p2win")
            nc.sync.dma_start(out=w64_sb[:], in_=w64_d[:])
            nc.sync.dma_start(out=iota_sb[:], in_=iota_d[:])
            nc.sync.dma_start(out=qidx_sb[:], in_=qidx_d[:])
            nc.sync.dma_start(out=p2win_sb[:], in_=p2win_d[:])

            # ---------------- phase 1: norms + feat table shard -------------
            with tc.tile_pool(name="p1", bufs=1) as p1:
                pS0 = p1.tile([P, NW], F32, tag="pS0")
                pS1 = p1.tile([P, NW], F32, tag="pS1")
                pD0 = p1.tile([P, NW], F32, tag="pD0")
                pD1 = p1.tile([P, NW], F32, tag="pD1")
                nc.sync.dma_start(out=pS0[:], in_=posS0_d[:])
                nc.sync.dma_start(out=pS1[:], in_=posS1_d[:])
                nc.sync.dma_start(out=pD0[:], in_=posD0_d[:])
                nc.sync.dma_start(out=pD1[:], in_=posD1_d[:])

                def rsqrt_diff(dstt, a1, a0, pool, pfx):
                    dg = pool.tile([P, NW], F32, tag=pfx + "dg")
                    nc.vector.tensor_tensor(
                        out=dg[:], in0=a1[:], in1=a0[:],
                        op=mybir.AluOpType.subtract,
                    )
                    cl = pool.tile([P, NW], F32, tag=pfx + "cl")
                    nc.vector.tensor_scalar_max(cl[:], dg[:], 1.0)
                    sq = pool.tile([P, NW], F32, tag=pfx + "sq")
                    nc.scalar.sqrt(sq[:], cl[:])
                    nc.vector.reciprocal(dstt[:], sq[:])

                normS = p1.tile([P, NW], F32, tag="normS")
                rsqrt_diff(normS, pS1, pS0, p1, "s")
                rsqrt_diff(normD, pD1, pD0, p1, "d")

                xb = p1.tile([P, NW, D], F32, tag="xb")
                nc.sync.dma_start(out=xb[:], in_=xs[:])
                fb = p1.tile([P, NW, EPAD], BF16, tag="fb")
                nc.vector.memset(fb[:, :, D:EPAD], 0.0)
                nc.vector.tensor_mul(
                    fb[:, :, 0:D], xb[:], normS[:, :, None].to_broadcast([P, NW, D])
                )
                nc.sync.dma_start(out=feat_s[:], in_=fb[:])

            # ---------------- allgather feat --------------------------------
            # Completion fence: Tile doesn't track the collective->gather RAW
            # dep through DRAM, so wait on an explicit semaphore inside a
            # critical section (Pool program order covers later gathers).
            ccsem = nc.alloc_semaphore("ccsem")
            with tc.tile_critical():
                nc.gpsimd.collective_compute(
                    "AllGather",
                    mybir.AluOpType.bypass,
                    replica_groups=[list(range(N_CORES))],
                    ins=[feat_s[:]],
                    outs=[feat_f[:]],
                ).then_inc(ccsem, 1)
                nc.gpsimd.wait_ge(ccsem, 1)

            # -------- phase 2: chunked gather + scatter matmul + W ----------
            with (
                tc.tile_pool(name="p2g", bufs=8) as p_g,
                tc.tile_pool(name="p2oh", bufs=3) as p_oh,
                tc.tile_pool(name="p2ps", bufs=4, space="PSUM") as p_ps,
                tc.tile_pool(name="p2ops", bufs=2, space="PSUM") as p_ops,
                tc.tile_pool(name="p2ag", bufs=2) as p_ag,
                tc.tile_pool(name="p2ob", bufs=2) as p_ob,
            ):

                def epilogue(w, ag):
                    op = p_ops.tile([P, D], F32)
                    nc.tensor.matmul(
                        out=op[:], lhsT=ag[:], rhs=w64_sb[:], start=True, stop=True
                    )
                    ob = p_ob.tile([P, D], F32, tag="ob")
                    nc.vector.tensor_mul(
                        ob[:], op[:], normD[:, w : w + 1].to_broadcast([P, D])
                    )
                    nb = min(P, SHARD - P * w)
                    nc.sync.dma_start(
                        out=out_d[P * w : P * w + nb, :], in_=ob[:nb, :]
                    )

                chunks = meta["chunks"]
                nidx_regs = {}
                ci = -1  # current chunk idx
                gb = None
                g0 = 0
                oh = None
                o0 = 0
                ps = None
                for t in range(T2):
                    if ci + 1 < len(chunks) and chunks[ci + 1][0] == t:
                        ci += 1
                        g0, ch, q = chunks[ci]
                        gb = p_g.tile([P, ch, EPAD], BF16, tag="gb")
                        qrows = min(QS, NFULL - QS * q)
                        n = P * ch
                        if n not in nidx_regs:
                            nidx_regs[n] = nc.gpsimd.to_reg(n)
                        nc.gpsimd.dma_gather(
                            out_ap=gb[:],
                            in_ap=feat_f[QS * q : QS * q + qrows, :],
                            idxs_ap=qidx_sb[:, 8 * g0 : 8 * (g0 + ch)],
                            num_idxs=n,
                            num_idxs_reg=nidx_regs[n],
                            elem_size=EPAD,
                            queue_num=ci % 4,
                        )
                    if t % CHO == 0:
                        o0 = t
                        co = min(CHO, T2 - t)
                        oh = p_oh.tile([P, co, W2], BF16, tag="oh")
                        nc.vector.tensor_tensor(
                            out=oh[:],
                            in0=p2win_sb[:, t : t + co, None].to_broadcast([P, co, W2]),
                            in1=iota_sb[:, None, :].to_broadcast([P, co, W2]),
                            op=mybir.AluOpType.is_equal,
                        )
                    w = meta["win_of_tile"][t]
                    q = meta["q_of_tile"][t]
                    if ps is None:
                        ps = p_ps.tile([D, P], F32)
                    nc.tensor.matmul(
                        out=ps[:],
                        lhsT=gb[:, t - g0, 0:D],
                        rhs=oh[:, t - o0, :],
                        start=(t == meta["seg_first"][(q, w)]),
                        stop=(t == meta["seg_last"][(q, w)]),
                    )
                    if t == meta["seg_last"][(q, w)]:
                        segs = meta["segs_of_win"][w]
                        if len(segs) == 1:
                            ag = p_ag.tile([D, P], BF16, tag="ag")
                            nc.vector.tensor_copy(ag[:], ps[:])
                            epilogue(w, ag)
                        elif q == segs[0]:
                            nc.vector.tensor_copy(acc[:, w, :], ps[:])
                        elif q != segs[-1]:
                            nc.vector.tensor_add(acc[:, w, :], ps[:], acc[:, w, :])
                        else:
                            ag = p_ag.tile([D, P], BF16, tag="ag")
                            nc.vector.tensor_add(ag[:], ps[:], acc[:, w, :])
                            epilogue(w, ag)
                        ps = None
                for w in meta["empty_wins"]:
                    ag = p_ag.tile([D, P], BF16, tag="ag")
                    nc.vector.memset(ag[:], 0.0)
                    epilogue(w, ag)

    from concourse.library_overlay import lower_extended_insts

    lower_extended_insts(nc)
    if do_split_waits:
        split_waits(nc)
    return nc


def kernel(x, W, src, dst):
    from concourse.bass_utils import run_bass_kernel_spmd

    ins_maps, meta = _prep(x, W, src, dst)
    meta = _tile_maps(meta)
    nc = _build_nc(meta)
    res = run_bass_kernel_spmd(nc, ins_maps, list(range(N_CORES)))
    out = np.concatenate([res.results[k]["out"] for k in range(N_CORES)], axis=0)
    return out.astype(np.float32)
